# revision 1
# baseline (speedup 1.0000x reference)
"""CrossAssetGNN forward on 8 Trainium2 cores, data-parallel over batch.

Key algebraic reductions vs the reference:
- Only the last 15 timesteps of x feed the output (causal dilated convs,
  receptive field 15, last-timestep readout) -> upload/compute 15/128 of x.
- The gather/scatter GAT over E=16770 random edges collapses to dense
  130x130 ops via a host-precomputed edge-count matrix C[dst,src]:
  every per-edge quantity depends only on (src,dst), so duplicate edges
  fold into integer multiplicities. The softmax max-subtraction cancels
  (up to the 1e-8 epsilon, ~1e-10 relative) and is dropped.
- Edge-weight MLP is evaluated densely for all (dst,src) pairs with the
  relu'd pre-activation block as the *stationary* matmul operand so each
  result column lands partition-parallel in PSUM.
"""
import json
import sys

sys.path.insert(0, "/opt/trn_rl_repo")

import numpy as np
from contextlib import ExitStack

import concourse.bass as bass
import concourse.tile as tile
from concourse import masks, mybir
from concourse.bass_utils import run_bass_kernel_spmd

f32 = mybir.dt.float32
AF = mybir.ActivationFunctionType
OP = mybir.AluOpType

B, A, AUX, T, DIN, H, ODIM = 32, 128, 2, 128, 64, 128, 3
N = A + AUX            # 130
NC_CORES = 8
NB = B // NC_CORES     # 4 graphs per core
W = 15                 # receptive field of the three causal convs
BN_EPS = 1e-5
NCOL = NB * N          # 520 node columns per core
JBLK = 26              # j's per dense edge-MLP block


# ---- walrus workaround: max 1 sync-wait command per instruction ------------
def _apply_sync_split_patch():
    if getattr(bass.Bass, "_sync_split_patched", False):
        return
    orig = bass.Bass.to_json_bytes

    def to_json_bytes(self, *a, **kw):
        m = json.loads(orig(self, *a, **kw))
        for f in m.get("functions", []):
            for blk in f.get("blocks", []):
                new = []
                for inst in blk.get("instructions", []):
                    si = inst.get("sync_info")
                    if (si and si.get("on_wait") and len(si["on_wait"]) > 1
                            and inst.get("engine") in
                            {"PE", "DVE", "Activation", "SP", "Pool"}):
                        waits = si["on_wait"]
                        for k, w in enumerate(waits[:-1]):
                            new.append({"engine": inst["engine"], "ins": [],
                                        "outs": [],
                                        "name": f"{inst['name']}-sw{k}",
                                        "opcode": "NoOp",
                                        "sync_info": {"on_update": [],
                                                      "on_wait": [w]}})
                        si["on_wait"] = waits[-1:]
                    new.append(inst)
                blk["instructions"] = new
        return json.dumps(m).encode()

    bass.Bass.to_json_bytes = to_json_bytes
    bass.Bass._sync_split_patched = True


def _bcast_ap(t, offset_elems, dims):
    """AP over SBUF tile t: partition dim + given free [step, count] dims."""
    return bass.AP(tensor=t.tensor, offset=t.offset + offset_elems,
                   ap=[list(t.ap[0])] + [list(d) for d in dims])


def _chunks(total, step):
    return [(s, min(step, total - s)) for s in range(0, total, step)]


def build_program():
    nc = bass.Bass("TRN2", target_bir_lowering=False, num_devices=NC_CORES)

    din = {}

    def d_in(name, shape):
        din[name] = nc.dram_tensor(name, list(shape), f32, kind="ExternalInput")
        return din[name]

    d_in("xt", [DIN, NCOL * W])
    d_in("W_embT", [DIN, H]); d_in("b_emb", [H, 1])
    d_in("cw_all", [H, 9 * H]); d_in("sc_all", [H, 3]); d_in("bi_all", [H, 3])
    d_in("W1aT", [H, H]); d_in("W1bT", [H, H]); d_in("b1", [H, 1])
    d_in("w2", [H, 1])
    d_in("CA", [128, N]); d_in("CB", [2, N])
    d_in("gWT", [H, 3 * H]); d_in("asrc", [H, 3]); d_in("adst", [H, 3])
    d_in("hW1T", [H, A * 64]); d_in("b1exp", [64, A * NB])
    d_in("hW2T", [64, A * ODIM]); d_in("b2exp", [ODIM, A * NB])
    d_in("b2ew", [1, 1])

    o_logits = nc.dram_tensor("logits", [ODIM, A * NB], f32, kind="ExternalOutput")
    o_probs = nc.dram_tensor("probs", [128, NB * ODIM], f32, kind="ExternalOutput")

    with tile.TileContext(nc) as tc:
        with ExitStack() as top:
            const = top.enter_context(tc.tile_pool(name="const", bufs=1))
            persist = top.enter_context(tc.tile_pool(name="persist", bufs=1))

            def load(name, shape):
                t = const.tile(list(shape), f32, name=f"c_{name}", tag=f"c_{name}")
                nc.sync.dma_start(out=t, in_=din[name][:, :])
                return t

            W_embT = load("W_embT", [DIN, H]); b_emb = load("b_emb", [H, 1])
            cw_all = load("cw_all", [H, 9 * H])
            sc_all = load("sc_all", [H, 3]); bi_all = load("bi_all", [H, 3])
            W1aT = load("W1aT", [H, H]); W1bT = load("W1bT", [H, H])
            b1 = load("b1", [H, 1]); w2 = load("w2", [H, 1])
            CAt = load("CA", [128, N]); CBt = load("CB", [2, N])
            gWT = load("gWT", [H, 3 * H])
            asrc = load("asrc", [H, 3]); adst = load("adst", [H, 3])
            hW1T = load("hW1T", [H, A * 64]); b1exp = load("b1exp", [64, A * NB])
            hW2T = load("hW2T", [64, A * ODIM]); b2exp = load("b2exp", [ODIM, A * NB])
            b2ap = din["b2ew"][:, :]
            b2col = const.tile([128, 1], f32)
            nc.sync.dma_start(out=b2col, in_=bass.AP(
                tensor=b2ap.tensor, offset=b2ap.offset, ap=[[0, 128], [1, 1]]))

            ident = const.tile([128, 128], f32)
            masks.make_identity(nc, ident[:, :])
            alpha02 = const.tile([128, 1], f32)
            nc.vector.memset(alpha02[:, :], 0.2)
            ones_row = const.tile([1, NCOL], f32)
            nc.vector.memset(ones_row[:, :], 1.0)

            feats = persist.tile([H, NCOL], f32)

            # ---------------- stage A: embed + 3 dilated causal convs -------
            with ExitStack() as sA:
                front = sA.enter_context(tc.tile_pool(name="front", bufs=1))
                psA = sA.enter_context(
                    tc.tile_pool(name="psA", bufs=3, space="PSUM"))

                xT = front.tile([DIN, NCOL * W], f32)
                nc.sync.dma_start(out=xT, in_=din["xt"][:, :])
                emb = front.tile([H, NCOL * W], f32)
                for s, ln in _chunks(NCOL * W, 512):
                    pe = psA.tile([128, 512], f32, tag="pe")
                    nc.tensor.matmul(pe[:, :ln], lhsT=W_embT[:, :],
                                     rhs=xT[:, s:s + ln], start=True, stop=True)
                    nc.scalar.activation(emb[:, s:s + ln], pe[:, :ln],
                                         AF.Identity, bias=b_emb[:, :])

                # conv layers: (out_len per block, in_len, dilation)
                l1 = front.tile([H, NCOL * 13], f32)
                l2 = front.tile([H, NCOL * 9], f32)
                convs = [(emb, W, 13, 1, 0, l1), (l1, 13, 9, 2, 1, l2),
                         (l2, 9, 1, 4, 2, feats)]
                for src, in_len, out_len, dil, li, dst in convs:
                    sv = src.rearrange("p (blk t) -> p blk t", t=in_len)
                    bpc = max(1, 507 // out_len)
                    for b0, nb in _chunks(NCOL, bpc):
                        pe = psA.tile([128, 512], f32, tag="pe")
                        w_cols = nb * out_len
                        for k in range(3):
                            rhs = sv[:, b0:b0 + nb,
                                     k * dil:k * dil + out_len]
                            nc.tensor.matmul(
                                pe[:, :w_cols],
                                lhsT=cw_all[:, (li * 3 + k) * H:(li * 3 + k + 1) * H],
                                rhs=rhs, start=(k == 0), stop=(k == 2))
                        nc.scalar.activation(
                            dst[:, b0 * out_len:b0 * out_len + w_cols],
                            pe[:, :w_cols], AF.Gelu,
                            bias=bi_all[:, li:li + 1], scale=sc_all[:, li:li + 1])

            # ---------------- stage B: dense edge-weight MLP ----------------
            ewA = [persist.tile([128, N], f32, name=f"ewA{b}", tag=f"ewA{b}")
                   for b in range(NB)]
            ewB = [persist.tile([2, N], f32, name=f"ewB{b}", tag=f"ewB{b}")
                   for b in range(NB)]
            with ExitStack() as sB:
                ewk = sB.enter_context(tc.tile_pool(name="ewk", bufs=3))
                psU = sB.enter_context(tc.tile_pool(name="psU", bufs=2, space="PSUM"))
                psE = sB.enter_context(tc.tile_pool(name="psE", bufs=2, space="PSUM"))

                Ut = persist.tile([H, NCOL], f32)
                Vt = persist.tile([H, NCOL], f32)
                for s, ln in _chunks(NCOL, 512):
                    pu = psU.tile([128, 512], f32, tag="uv")
                    nc.tensor.matmul(pu[:, :ln], lhsT=W1aT[:, :],
                                     rhs=feats[:, s:s + ln], start=True, stop=True)
                    nc.vector.tensor_copy(Ut[:, s:s + ln], pu[:, :ln])
                    pv = psU.tile([128, 512], f32, tag="uv")
                    nc.tensor.matmul(pv[:, :ln], lhsT=W1bT[:, :],
                                     rhs=feats[:, s:s + ln], start=True, stop=True)
                    nc.scalar.activation(Vt[:, s:s + ln], pv[:, :ln],
                                         AF.Identity, bias=b1[:, :])

                for b in range(NB):
                    pA = psE.tile([128, N], f32, tag="ewpsA")
                    pB = psE.tile([2, N], f32, tag="ewpsB")
                    for jb in range(N // JBLK):
                        R = ewk.tile([128, JBLK * N], f32, tag="R")
                        in0 = _bcast_ap(Ut, b * N + jb * JBLK, [[1, JBLK], [0, N]])
                        in1 = _bcast_ap(Vt, b * N, [[0, JBLK], [1, N]])
                        nc.vector.tensor_tensor(out=R[:, :], in0=in0, in1=in1,
                                                op=OP.add)
                        nc.scalar.activation(R[:, :], R[:, :], AF.Relu)
                        for jl in range(JBLK):
                            j = jb * JBLK + jl
                            nc.tensor.matmul(pA[:, j:j + 1],
                                             lhsT=R[:, jl * N:jl * N + 128],
                                             rhs=w2[:, :], start=True, stop=True)
                            nc.tensor.matmul(pB[:, j:j + 1],
                                             lhsT=R[:, jl * N + 128:jl * N + N],
                                             rhs=w2[:, :], start=True, stop=True)
                    nc.scalar.activation(ewA[b][:, :], pA[:, :], AF.Sigmoid,
                                         bias=b2col[:, :])
                    nc.scalar.activation(ewB[b][:, :], pB[:, :], AF.Sigmoid,
                                         bias=b2col[0:2, :])

            # ---------------- stage C: 3 dense GAT layers -------------------
            nfT = feats
            with ExitStack() as sC:
                gw = sC.enter_context(tc.tile_pool(name="gw", bufs=2))
                gps = sC.enter_context(tc.tile_pool(name="gps", bufs=1, space="PSUM"))
                gsq = sC.enter_context(tc.tile_pool(name="gsq", bufs=2, space="PSUM"))

                for li in range(3):
                    gW = gWT[:, li * H:(li + 1) * H]
                    hpT = gw.tile([H, NCOL], f32, tag="hpT")
                    for s, ln in _chunks(NCOL, 512):
                        ph = gps.tile([128, 512], f32, tag="big")
                        nc.tensor.matmul(ph[:, :ln], lhsT=gW, rhs=nfT[:, s:s + ln],
                                         start=True, stop=True)
                        nc.vector.tensor_copy(hpT[:, s:s + ln], ph[:, :ln])

                    as_sb = gw.tile([1, NCOL], f32, tag="as")
                    ad_sb = gw.tile([1, NCOL], f32, tag="ad")
                    for col, vec, dst in ((0, asrc, as_sb), (1, adst, ad_sb)):
                        pav = gsq.tile([1, NCOL], f32, tag="arow", bufs=1)
                        for s, ln in _chunks(NCOL, 512):
                            nc.tensor.matmul(pav[0:1, s:s + ln],
                                             lhsT=vec[:, li:li + 1],
                                             rhs=hpT[:, s:s + ln],
                                             start=True, stop=True)
                        nc.vector.tensor_copy(dst[:, :], pav[:, :])

                    R2 = gw.tile([2, NCOL], f32, tag="R2")
                    nc.vector.memset(R2[0:1, :], 1.0)
                    nc.sync.dma_start(out=R2[1:2, :], in_=as_sb[:, :])

                    hpA, hpB = [], []
                    for b in range(NB):
                        pn = gsq.tile([128, 128], f32, tag="sq")
                        nc.tensor.matmul(pn[:, :], lhsT=nfT[:, b * N:b * N + 128],
                                         rhs=gW, start=True, stop=True)
                        ha = gw.tile([128, H], f32, name=f"hpA{b}", tag=f"hpA{b}")
                        nc.vector.tensor_copy(ha[:, :], pn[:, :])
                        hpA.append(ha)
                        pn2 = gsq.tile([2, 128], f32, tag="tiny")
                        nc.tensor.matmul(pn2[:, :], lhsT=nfT[:, b * N + 128:b * N + N],
                                         rhs=gW, start=True, stop=True)
                        hb = gw.tile([2, H], f32, name=f"hpB{b}", tag=f"hpB{b}")
                        nc.vector.tensor_copy(hb[:, :], pn2[:, :])
                        hpB.append(hb)

                    nfT_next = gw.tile([H, NCOL], f32, tag="nfT")
                    for b in range(NB):
                        L2b = gw.tile([2, N], f32, tag="L2b")
                        nc.vector.tensor_copy(L2b[0:1, :], ad_sb[0:1, b * N:(b + 1) * N])
                        nc.sync.dma_start(out=L2b[1:2, :], in_=ones_row[0:1, 0:N])

                        pa = gsq.tile([128, N], f32, tag="sq")
                        nc.tensor.matmul(pa[:, :N], lhsT=L2b[:, 0:128],
                                         rhs=R2[:, b * N:(b + 1) * N],
                                         start=True, stop=True)
                        pb = gsq.tile([2, N], f32, tag="tiny")
                        nc.tensor.matmul(pb[:, :N], lhsT=L2b[:, 128:N],
                                         rhs=R2[:, b * N:(b + 1) * N],
                                         start=True, stop=True)

                        PA = gw.tile([128, N], f32, tag="PA")
                        PB = gw.tile([2, N], f32, tag="PB")
                        sA_ = gw.tile([128, 1], f32, tag="sA")
                        sB_ = gw.tile([2, 1], f32, tag="sB")
                        for (pp, ew, Ct, Pt, st, rows) in (
                                (pa, ewA[b], CAt, PA, sA_, 128),
                                (pb, ewB[b], CBt, PB, sB_, 2)):
                            t_ = gw.tile([rows, N], f32, tag=f"t{rows}")
                            nc.scalar.activation(t_[:, :], pp[:rows, :N], AF.Prelu,
                                                 alpha=alpha02[:rows, :])
                            z_ = gw.tile([rows, N], f32, tag=f"z{rows}")
                            nc.vector.tensor_tensor(out=z_[:, :], in0=t_[:, :],
                                                    in1=ew[:, :], op=OP.mult)
                            e_ = gw.tile([rows, N], f32, tag=f"e{rows}")
                            nc.scalar.activation(e_[:, :], z_[:, :], AF.Exp)
                            nc.vector.scalar_tensor_tensor(
                                out=Pt[:, :], in0=e_[:, :], scalar=1.0,
                                in1=Ct[:, :], op0=OP.mult, op1=OP.mult,
                                accum_out=st[:, :])

                        rA = gw.tile([128, 1], f32, tag="rA")
                        rAn = gw.tile([128, 1], f32, tag="rAn")
                        rB = gw.tile([2, 1], f32, tag="rB")
                        rBn = gw.tile([2, 1], f32, tag="rBn")
                        for st, rr, rn in ((sA_, rA, rAn), (sB_, rB, rBn)):
                            nc.vector.tensor_scalar_add(st[:, :], st[:, :], 1e-8)
                            nc.vector.reciprocal(rr[:, :], st[:, :])
                            nc.vector.tensor_scalar_mul(rn[:, :], rr[:, :], -1.0)

                        # transpose P -> PT (src-major) for the aggregation
                        PT = gw.tile([128, N], f32, tag="PT")
                        PT2 = gw.tile([2, N], f32, tag="PT2")
                        pt1 = gsq.tile([128, 128], f32, tag="sq")
                        nc.tensor.transpose(pt1[:, :], PA[:, 0:128], ident[:, :])
                        nc.vector.tensor_copy(PT[:, 0:128], pt1[:, :])
                        pt2 = gsq.tile([2, 128], f32, tag="tiny")
                        nc.tensor.transpose(pt2[:, :], PA[:, 128:N], ident[:, :])
                        nc.vector.tensor_copy(PT2[:, 0:128], pt2[:, :])
                        pt3 = gsq.tile([128, 2], f32, tag="col2", bufs=1)
                        nc.tensor.transpose(pt3[:, :], PB[:, 0:128], ident[0:2, 0:2])
                        nc.vector.tensor_copy(PT[:, 128:N], pt3[:, :])
                        pt4 = gsq.tile([2, 2], f32, tag="tiny")
                        nc.tensor.transpose(pt4[:, :], PB[:, 128:N], ident[0:2, 0:2])
                        nc.vector.tensor_copy(PT2[:, 128:N], pt4[:, :])

                        po = gsq.tile([128, H], f32, tag="sq")
                        nc.tensor.matmul(po[:, :], lhsT=PT[:, 0:128], rhs=hpA[b][:, :],
                                         start=True, stop=False)
                        nc.tensor.matmul(po[:, :], lhsT=PT2[:, 0:128], rhs=hpB[b][:, :],
                                         start=False, stop=True)
                        po2 = gsq.tile([2, H], f32, tag="tiny")
                        nc.tensor.matmul(po2[:, :], lhsT=PT[:, 128:N], rhs=hpA[b][:, :],
                                         start=True, stop=False)
                        nc.tensor.matmul(po2[:, :], lhsT=PT2[:, 128:N], rhs=hpB[b][:, :],
                                         start=False, stop=True)

                        # elu(out * r) eviction, then transpose back to feat-major
                        for (pp, rr, rn, rows, coff) in (
                                (po, rA, rAn, 128, 0), (po2, rB, rBn, 2, 128)):
                            pos = gw.tile([rows, H], f32, tag=f"pos{rows}")
                            nc.scalar.activation(pos[:, :], pp[:rows, :], AF.Relu,
                                                 scale=rr[:rows, :])
                            neg = gw.tile([rows, H], f32, tag=f"neg{rows}")
                            nc.scalar.activation(neg[:, :], pp[:rows, :], AF.Relu,
                                                 scale=rn[:rows, :])
                            ex = gw.tile([rows, H], f32, tag=f"ex{rows}")
                            nc.scalar.activation(ex[:, :], neg[:, :], AF.Exp,
                                                 scale=-1.0)
                            nf_ = gw.tile([rows, H], f32, tag=f"nf{rows}")
                            nc.vector.scalar_tensor_tensor(
                                out=nf_[:, :], in0=ex[:, :], scalar=1.0,
                                in1=pos[:, :], op0=OP.subtract, op1=OP.add)
                            if rows == 128:
                                ptb = gsq.tile([128, 128], f32, tag="sq")
                                nc.tensor.transpose(ptb[:, :], nf_[:, :], ident[:, :])
                                nc.vector.tensor_copy(
                                    nfT_next[:, b * N:b * N + 128], ptb[:, :])
                            else:
                                ptb = gsq.tile([128, 2], f32, tag="col2", bufs=1)
                                nc.tensor.transpose(ptb[:, :], nf_[:, :],
                                                    ident[0:2, 0:2])
                                nc.vector.tensor_copy(
                                    nfT_next[:, b * N + 128:b * N + N], ptb[:, :])
                    nfT = nfT_next

            # ---------------- stage D: per-asset heads + softmax ------------
            with ExitStack() as sD:
                hw = sD.enter_context(tc.tile_pool(name="hw", bufs=1))
                hps = sD.enter_context(tc.tile_pool(name="hps", bufs=1, space="PSUM"))
                hsq = sD.enter_context(tc.tile_pool(name="hsq", bufs=4, space="PSUM"))

                hid_ps = hps.tile([64, A * NB], f32, tag="hid")
                for a in range(A):
                    rhs = bass.AP(tensor=nfT.tensor, offset=nfT.offset + a,
                                  ap=[list(nfT.ap[0]), [N, NB]])
                    nc.tensor.matmul(hid_ps[:, a * NB:(a + 1) * NB],
                                     lhsT=hW1T[:, a * 64:(a + 1) * 64],
                                     rhs=rhs, start=True, stop=True)
                hid = hw.tile([64, A * NB], f32)
                nc.vector.tensor_tensor(out=hid[:, :], in0=hid_ps[:, :],
                                        in1=b1exp[:, :], op=OP.add)
                nc.scalar.activation(hid[:, :], hid[:, :], AF.Relu)

                log_ps = hps.tile([ODIM, A * NB], f32, tag="log")
                for a in range(A):
                    nc.tensor.matmul(log_ps[:, a * NB:(a + 1) * NB],
                                     lhsT=hW2T[:, a * ODIM:(a + 1) * ODIM],
                                     rhs=hid[:, a * NB:(a + 1) * NB],
                                     start=True, stop=True)
                logits = hw.tile([ODIM, A * NB], f32)
                nc.vector.tensor_tensor(out=logits[:, :], in0=log_ps[:, :],
                                        in1=b2exp[:, :], op=OP.add)
                nc.sync.dma_start(out=o_logits[:, :], in_=logits[:, :])

                # softmax over ODIM: transpose to (128, 4, 3), exp on eviction
                e_sb = hw.tile([128, NB * ODIM], f32)
                for c in range(NB):
                    pt = hsq.tile([128, ODIM], f32, tag="sm")
                    nc.tensor.transpose(pt[:, :], logits[:, c * 128:(c + 1) * 128],
                                        ident[0:ODIM, 0:ODIM])
                    nc.scalar.activation(e_sb[:, c * ODIM:(c + 1) * ODIM],
                                         pt[:, :], AF.Exp)
                s_sb = hw.tile([128, NB], f32)
                for c in range(NB):
                    nc.vector.tensor_tensor(out=s_sb[:, c:c + 1],
                                            in0=e_sb[:, c * ODIM:c * ODIM + 1],
                                            in1=e_sb[:, c * ODIM + 1:c * ODIM + 2],
                                            op=OP.add)
                    nc.vector.tensor_tensor(out=s_sb[:, c:c + 1],
                                            in0=s_sb[:, c:c + 1],
                                            in1=e_sb[:, c * ODIM + 2:c * ODIM + 3],
                                            op=OP.add)
                r_sb = hw.tile([128, NB], f32)
                nc.vector.reciprocal(r_sb[:, :], s_sb[:, :])
                probs = hw.tile([128, NB * ODIM], f32)
                r_b = _bcast_ap(r_sb, 0, [[1, NB], [0, ODIM]])
                nc.vector.tensor_tensor(out=probs[:, :], in0=e_sb[:, :],
                                        in1=r_b, op=OP.mult)
                nc.sync.dma_start(out=o_probs[:, :], in_=probs[:, :])

    return nc


def host_inputs(x, edge_index, W_emb, b_emb, conv_w, conv_b, bn_gamma, bn_beta,
                bn_mean, bn_var, gat_W, gat_a_src, gat_a_dst, ew_W1, ew_b1,
                ew_W2, ew_b2, head_W1, head_b1, head_W2, head_b2):
    """Per-core input dicts (host-side preprocessing)."""
    f = np.float32
    xs = np.asarray(x, f)[:, :, T - W:, :]                       # (B,N,15,64)
    xt = np.ascontiguousarray(np.transpose(xs, (3, 0, 1, 2)))    # (64,B,N,15)

    ei = np.asarray(edge_index)
    C = np.zeros((N, N), f)
    np.add.at(C, (ei[1].astype(np.int64), ei[0].astype(np.int64)), 1.0)

    inv = np.asarray(bn_gamma, f) / np.sqrt(np.asarray(bn_var, f) + BN_EPS)
    sc_all = inv.T.copy()                                        # (H,3)
    bi_all = ((np.asarray(conv_b, f) - np.asarray(bn_mean, f)) * inv
              + np.asarray(bn_beta, f)).T.copy()                 # (H,3)
    cw = np.asarray(conv_w, f)                                   # (3,H,H,3)
    cw_all = np.concatenate(
        [cw[i, :, :, k].T for i in range(3) for k in range(3)], axis=1)

    ew_W1 = np.asarray(ew_W1, f)
    gat_W = np.asarray(gat_W, f)
    hW1 = np.asarray(head_W1, f); hW2 = np.asarray(head_W2, f)
    # b1exp[k, a*NB+bi] = head_b1[a,k]
    b1exp = np.repeat(np.asarray(head_b1, f).T[:, :, None], NB, axis=2)
    b1exp = b1exp.reshape(64, A * NB)
    b2exp = np.repeat(np.asarray(head_b2, f).T[:, :, None], NB, axis=2)
    b2exp = b2exp.reshape(ODIM, A * NB)

    shared = {
        "W_embT": np.ascontiguousarray(np.asarray(W_emb, f).T),
        "b_emb": np.asarray(b_emb, f).reshape(H, 1),
        "cw_all": np.ascontiguousarray(cw_all),
        "sc_all": np.ascontiguousarray(sc_all),
        "bi_all": np.ascontiguousarray(bi_all),
        "W1aT": np.ascontiguousarray(ew_W1[:, :H].T),
        "W1bT": np.ascontiguousarray(ew_W1[:, H:].T),
        "b1": np.asarray(ew_b1, f).reshape(H, 1),
        "w2": np.ascontiguousarray(np.asarray(ew_W2, f).reshape(1, H).T),
        "b2ew": np.asarray(ew_b2, f).reshape(1, 1),
        "CA": np.ascontiguousarray(C[:128]),
        "CB": np.ascontiguousarray(C[128:]),
        "gWT": np.ascontiguousarray(
            np.concatenate([gat_W[i].T for i in range(3)], axis=1)),
        "asrc": np.ascontiguousarray(
            np.stack([np.asarray(gat_a_src, f)[i, 0] for i in range(3)], axis=1)),
        "adst": np.ascontiguousarray(
            np.stack([np.asarray(gat_a_dst, f)[i, 0] for i in range(3)], axis=1)),
        "hW1T": np.ascontiguousarray(
            np.concatenate([hW1[a].T for a in range(A)], axis=1)),
        "b1exp": np.ascontiguousarray(b1exp),
        "hW2T": np.ascontiguousarray(
            np.concatenate([hW2[a].T for a in range(A)], axis=1)),
        "b2exp": np.ascontiguousarray(b2exp),
    }
    in_maps = []
    for c in range(NC_CORES):
        m = dict(shared)
        m["xt"] = np.ascontiguousarray(
            xt[:, c * NB:(c + 1) * NB].reshape(DIN, NCOL * W))
        in_maps.append(m)
    return in_maps


_CACHE = {}


def kernel(**inputs):
    _apply_sync_split_patch()
    if "nc" not in _CACHE:
        _CACHE["nc"] = build_program()
    nc = _CACHE["nc"]
    in_maps = host_inputs(**inputs)
    res = run_bass_kernel_spmd(nc, in_maps, list(range(NC_CORES)), trace=False)
    logits = np.empty((B, A, ODIM), np.float32)
    probs = np.empty((B, A, ODIM), np.float32)
    for c in range(NC_CORES):
        lg = res.results[c]["logits"]          # (3, A*NB)
        pr = res.results[c]["probs"]           # (128, NB*3)
        logits[c * NB:(c + 1) * NB] = lg.reshape(ODIM, A, NB).transpose(2, 1, 0)
        # probs rows: chunk c2 covers logit cols c2*128..; col idx = a*NB+bi
        tmp = pr.reshape(128, NB, ODIM).transpose(1, 0, 2).reshape(A * NB, ODIM)
        probs[c * NB:(c + 1) * NB] = tmp.reshape(A, NB, ODIM).transpose(1, 0, 2)
    return logits, probs

